# revision 6
# baseline (speedup 1.0000x reference)
"""Trainium2 Bass kernel for a 2-layer GCN (segment-sum aggregation).

out = softmax( A @ relu(A @ h @ W1 + b1) @ W2 + b2 ),  A = adjacency (+self loops)

Strategy (8 NeuronCores, node/data parallel), ~1.32 ms vs 2.15 ms for the
gather-both-layers baseline:
  - Nodes sharded by range: core k owns nodes [k*12500, (k+1)*12500).
  - LAYER 1 (l1red): the host pre-gathers h[src] per edge into a dense
    dst-slotted stream (slot (window, dst, j) = j-th in-edge; self-loop is
    the last slot; zero padding). Window block layout
    [128 = feat + 64*(dst>=64), 64 dst, nslot_w] bf16 makes the on-device
    segment-sum a single strided tensor_reduce per window (no gathers, no
    one-hots in layer 1). Then two half-window matmuls vs a partition-
    stacked W1 (PE tile_position 64), relu via ACT with bias, y = x1T.T@W2.
    (Writing both matmul halves into one PSUM tile at column offsets
    crashed HW; separate PSUM tiles + two activations work.)
  - AllGather of per-core y slices (64 cols bf16, cc40=False: the 40-col
    variant + on-device expand measured slower) -> full y gather table,
    chunked 4x (cc_chunks) so 3/4 of the traffic overlaps layer-1 compute;
    only the last quarter blocks layer 2.
  - LAYER 2: dma_gather of y rows (256B elems from the [N/4, 1024B]
    super-row table; src%4 phase slots dodge the signed-int16 index limit);
    per 128-edge chunk a one-hot [edge x dst] matrix is built on VectorE
    (is_equal vs iota, -1000 marks padding) and TensorE accumulates into
    PSUM per window; + own-y self loop + b2; softmax on chip.
  - Layer-2 slots are exact-max over cores (no 128 rounding): 19% fewer
    gather positions; chunks spanning window boundaries get one masked
    one-hot column per window. Slot runs share the table half across
    adjacent phases -> one gather instruction per (group, phase-pair),
    maxidx=4096 (fewer, larger gathers measured faster than 1024).
    HW gathers measured BYTE-bound (512B elems were slower), so indices
    are src-sorted per slot for HBM locality and stay at 256B.
"""

import math
import numpy as np

D = 64          # input feature dim (one gather row = 256B)
HID = 128
C = 40
CORES = 8
WIN = 128       # dst window (nodes per one-hot matmul window)
NPHASE = 4      # src mod-4 phases (int16 gather index reach)
GROUP = 4       # windows per gather instruction group


# ----------------------------------------------------------------------------
# Host-side routing
# ----------------------------------------------------------------------------

def cc_chunk_bounds(nw, nloc, q, group=GROUP, geo=False):
    """Split windows into q chunks -> list of (a, b) local-node row ranges.

    Boundaries land on gather-group multiples so the chunked AllGather's
    trigger (last window of a group) actually fires for every chunk.
    geo=True halves chunk sizes geometrically so the last (blocking)
    AllGather before layer 2 is tiny.
    """
    res = []
    w0 = 0
    for i in range(q):
        if i == q - 1:
            w1 = nw
        elif geo:
            w1 = max(w0 + group,
                     int(nw * (1.0 - 0.5 ** (i + 1))) // group * group)
        else:
            w1 = ((nw * (i + 1)) // (q * group)) * group
        w1 = min(w1, nw)
        res.append((w0 * WIN, min(w1 * WIN, nloc)))
        w0 = w1
    return res


def route_edges(src, dst, n_nodes, cores=CORES, cc_chunks=1, group=GROUP,
                cc_geo=False):
    """Group edges by (core, window, phase) into exact-max static slots.

    Slot (w, phi) holds max-over-cores count of matching edges (no 128
    rounding). Per-(group, phase) runs are concatenated and their chunk
    grid padded to 128; a chunk spanning a window boundary gets one masked
    one-hot column per window. Trailing run padding is skipped by the
    gather via negative indices.
    """
    nloc = n_nodes // cores
    nw = math.ceil(nloc / WIN)
    src = src.astype(np.int64)
    dst = dst.astype(np.int64)
    core = dst // nloc
    dloc = dst % nloc
    w = dloc // WIN
    phi = src % NPHASE
    key = (w * NPHASE + phi).astype(np.int64)   # per-core key in [0, nw*4)

    counts = np.zeros((cores, nw * NPHASE), np.int64)
    for k in range(cores):
        counts[k] = np.bincount(key[core == k], minlength=nw * NPHASE)
    S = counts.max(axis=0)                      # [nw*4] exact-max slots
    # ensure every window has at least one position (avoids empty PSUM)
    for wi in range(nw):
        if S[wi * NPHASE:(wi + 1) * NPHASE].sum() == 0:
            S[wi * NPHASE] = 1

    # stream order: for group g: for phi: for w in group: slot(w, phi);
    # each (g, phi) run is padded to a whole number of 128-row chunks.
    ngroups = math.ceil(nw / group)
    offs = np.zeros(nw * NPHASE, np.int64)      # slot -> stream offset
    runs = []            # (g, p, base_pos, n_real) per run
    wchunks = {wi: [] for wi in range(nw)}      # wi -> [(dcol, gcol, half)]
    gchunks_l = [0] * ngroups                   # chunks per group
    ndcol = 0                                   # one-hot columns total
    dcol_fill = []       # (dcol, gcol, wi, lo, hi) row spans per column
    pos = 0
    for g in range(ngroups):
        ws = list(range(g * group, min((g + 1) * group, nw)))
        for p in range(NPHASE):
            base = pos
            n_real = 0
            bounds = []                          # (wi, lo, hi) within run
            for wi in ws:
                s = int(S[wi * NPHASE + p])
                offs[wi * NPHASE + p] = base + n_real
                if s:
                    bounds.append((wi, n_real, n_real + s))
                n_real += s
            nchk = (n_real + WIN - 1) // WIN
            runs.append((g, p, base, n_real))
            for c in range(nchk):
                gcol = base // WIN + c
                lo, hi = c * WIN, (c + 1) * WIN
                for (wi, a, b) in bounds:
                    if a < hi and b > lo:
                        wchunks[wi].append((ndcol, gcol, p))
                        dcol_fill.append((ndcol, gcol,
                                          base + max(a, lo),
                                          base + min(b, hi)))
                        ndcol += 1
            gchunks_l[g] += nchk
            pos += nchk * WIN
    tot = int(pos)

    # y4 super-row index per node: rank-major (cc_chunks==1) or
    # chunk-major/rank/local (chunked AllGather writes y4 chunk by chunk)
    if cc_chunks > 1:
        bounds = cc_chunk_bounds(nw, nloc, cc_chunks, group, cc_geo)
        aqs = np.array([a for a, b in bounds], np.int64)
        rows = np.array([b - a for a, b in bounds], np.int64)

        def y4row(n):
            c, loc = n // nloc, n % nloc
            q = np.searchsorted(aqs, loc, side="right") - 1
            return 2 * aqs[q] + c * (rows[q] // 4) + (loc - aqs[q]) // 4
    else:
        bounds = None

        def y4row(n):
            return n >> 2



    idx_streams, idx2_streams, dst_streams, src_streams = [], [], [], []
    for k in range(cores):
        sel = core == k
        kk = key[sel]
        # secondary sort by src: gather descriptors walk the table in
        # ascending address order (HBM row-buffer locality)
        sidx = np.lexsort((src[sel], kk))
        kk_s = kk[sidx]
        # occurrence rank within each key group
        occ = np.arange(len(kk_s)) - np.repeat(
            np.r_[0, np.cumsum(np.bincount(kk_s, minlength=nw * NPHASE))[:-1]][kk_s], 1)
        pos_k = offs[kk_s] + occ
        idx = np.zeros(tot, np.int16)           # pad: super-row 0 (valid)
        idx2 = np.zeros(tot, np.int16)
        dsl = np.full(tot, -1000.0, np.float32)  # pad: no one-hot match
        srplaces = np.zeros(tot, np.int64)      # pad: node 0 (masked anyway)
        srt = src[sel][sidx]
        idx[pos_k] = (srt >> 2).astype(np.int16)
        idx2[pos_k] = y4row(srt).astype(np.int16)
        dsl[pos_k] = (dloc[sel][sidx] % WIN).astype(np.float32)
        srplaces[pos_k] = srt
        # one-hot column table [ndcol, WIN]
        dstf = np.full((ndcol, WIN), -1000.0, np.float32)
        for (dcol, gcol, ga, gb) in dcol_fill:
            r0, r1 = ga - gcol * WIN, gb - gcol * WIN
            dstf[dcol, r0:r1] = dsl[ga:gb]
        idx_streams.append(idx)
        idx2_streams.append(idx2)
        dst_streams.append(dstf)
        src_streams.append(srplaces)
    return dict(S=S, offs=offs, tot=tot, nw=nw, nloc=nloc,
                ngroups=ngroups, idx=idx_streams, idx2=idx2_streams,
                dst=dst_streams, srcs=src_streams,
                runs=runs, wchunks=wchunks, gch=gchunks_l, ndcol=ndcol,
                cc_chunks=cc_chunks, cc_bounds=bounds,
                group=group)


def route_l1(src, dst, h_bf, n_nodes, cores=CORES):
    """Layer-1 pre-gathered streams for the reduce scheme: per core a
    [128, sum_w 64*nslot_w] bf16 array; window w block is
    [128 = feat + 64*(dst>=64), 64 dst, nslot_w] with slot j = j-th
    in-edge's h[src] row (self-loop last; zero slots pad)."""
    nloc = n_nodes // cores
    nw = math.ceil(nloc / WIN)
    nlocp = nw * WIN
    src = src.astype(np.int64)
    dst = dst.astype(np.int64)
    core = dst // nloc
    dloc = dst % nloc

    degs = np.zeros((cores, nlocp), np.int64)
    for k in range(cores):
        degs[k, :nloc] = np.bincount(dloc[core == k], minlength=nloc)
    dmax = degs.reshape(cores, nw, WIN).max(axis=(0, 2))
    nslot = np.maximum(dmax + 1, 2).astype(np.int64)     # [nw] +self
    ncols = int(D * nslot.sum())

    streams = []
    for k in range(cores):
        sel = core == k
        dl = dloc[sel]
        sidx = np.argsort(dl, kind="stable")
        dl_s = dl[sidx]
        cnts = np.bincount(dl_s, minlength=nlocp)
        starts = np.r_[0, np.cumsum(cnts)[:-1]]
        occ = np.arange(len(dl_s)) - starts[dl_s]
        rows = h_bf[src[sel][sidx]]                      # [Ek, 64]
        out = np.zeros((128, ncols), h_bf.dtype)
        col = 0
        bw = np.searchsorted(dl_s, np.arange(0, nlocp + 1, WIN))
        for w in range(nw):
            ns = int(nslot[w])
            arr = np.zeros((ns, WIN, D), h_bf.dtype)
            a, b = bw[w], bw[w + 1]
            arr[occ[a:b], dl_s[a:b] - w * WIN] = rows[a:b]
            nrows = min(WIN, nloc - w * WIN)             # self-loops
            dvec = np.arange(nrows)
            arr[cnts[w * WIN + dvec], dvec] = \
                h_bf[k * nloc + w * WIN + dvec]
            # [j, dhi, dlo, f] -> [dhi, f, dlo, j] -> [128, 64*ns]
            t = arr.reshape(ns, 2, D, D).transpose(1, 3, 2, 0)
            out[:, col:col + D * ns] = t.reshape(128, D * ns)
            col += D * ns
        streams.append(out)
    return dict(nslot=nslot, ncols=ncols, streams=streams)


# ----------------------------------------------------------------------------
# Bass program
# ----------------------------------------------------------------------------

def build_program(n_nodes, rt, rt1=None, do_cc=True, l2_table_y=True,
                  maxidx=1024,
                  scratch=16384, skip_compute=False, skip_gather=False,
                  elem512=False, gbufs=3, single_packet=True, cc_chunks=1,
                  cc40=False, group=GROUP, l1red=False, cc_geo=False,
                  deep=False, l1group=None):
    import concourse.bass as bass
    import concourse.mybir as mybir
    import concourse.bacc as bacc
    from concourse import tile

    f32 = mybir.dt.float32
    bf16 = mybir.dt.float16
    i16 = mybir.dt.int16
    S, offs, tot, nw, nloc, ngroups = (rt["S"], rt["offs"], rt["tot"],
                                       rt["nw"], rt["nloc"], rt["ngroups"])
    nch = tot // WIN                       # total chunks
    nsup = n_nodes // NPHASE               # super-rows in gather tables
    nlocp = nw * WIN                       # padded local node count
    last_rows = nloc - (nw - 1) * WIN      # rows in the last window

    nc = bacc.Bacc(None, target_bir_lowering=False, debug=False,
                   num_swdge_queues=4, dynamic_dma_scratch_size=scratch)

    if l1red:
        # layer-1 stream in window-block layout for the reduce scheme
        l1rd = nc.declare_dram_parameter("l1r", [128, rt1["ncols"]], bf16,
                                         False)
        nslot = rt1["nslot"]
        l1off = np.zeros(nw + 1, np.int64)
        l1off[1:] = np.cumsum(D * nslot)
    else:
        # layer-1 pre-gathered edge stream: chunk-major [WIN, nch, D] bf16
        l1sd = nc.declare_dram_parameter("l1s", [128, (tot // WIN) * D],
                                         bf16, False)
        hTo = nc.declare_dram_parameter("hTo", [D, nlocp], f32, False)
    W1d = nc.declare_dram_parameter("W1", [2 * D, HID], f32, False)
    b1d = nc.declare_dram_parameter("b1", [HID, 1], f32, False)
    W2d = nc.declare_dram_parameter("W2p", [HID, D], f32, False)
    b2d = nc.declare_dram_parameter("b2b", [WIN, D], f32, False)
    # the layer-1 h-table index stream is only needed when it doubles as
    # the layer-2 stream (cc_chunks == 1: y4row == src >> 2)
    need_idx1 = cc_chunks == 1 or not l1red
    idxd = (nc.declare_dram_parameter("idx", [128, tot // 16], i16, False)
            if need_idx1 else None)
    idx2d = (nc.declare_dram_parameter("idx2", [128, tot // 16], i16, False)
             if cc_chunks > 1 else None)
    dstd = nc.declare_dram_parameter("dstf", [WIN, rt["ndcol"]], f32, False)
    iotad = nc.declare_dram_parameter("iota", [WIN, WIN], bf16, False)
    outd = nc.declare_dram_parameter("out", [nloc, C], f32, True)

    ccd = C if cc40 else D                 # cols moved by the AllGather
    cc_in = nc.dram_tensor("cc_in", [nloc, ccd], bf16)
    if cc40:
        # gather table arrives as a pre-zeroed input; expand_y40 fills :C
        y4 = nc.declare_dram_parameter(
            "y4z", [CORES * nloc // NPHASE, NPHASE * D], bf16, False)
        y40 = nc.dram_tensor("y40", [CORES * nloc, C], bf16,
                             addr_space="Shared")
    else:
        y4 = nc.dram_tensor("y4", [CORES * nloc // NPHASE, NPHASE * D], bf16,
                            addr_space="Shared")
        y40 = None

    def expand_y40(a, b):
        # pad 40-col allgathered rows into the 64-col/node gather table
        # (batched: walrus caps one AP dim at 65535 elements)
        step = 8192 * NPHASE            # nodes per DMA (8192 super-rows)
        for n0 in range(CORES * a, CORES * b, step):
            n1 = min(n0 + step, CORES * b)
            nc.sync.dma_start(
                y4[n0 // NPHASE:n1 // NPHASE, :]
                .rearrange("r (n f) -> r n f", n=NPHASE)[:, :, :C],
                y40[n0:n1, :].rearrange("(r n) f -> r n f", n=NPHASE))

    # slot geometry helpers -------------------------------------------------
    def group_windows(g):
        return range(g * group, min((g + 1) * group, nw))

    gchunks = rt["gch"]                    # chunks per group
    colbase = [0]                          # group -> first global chunk
    for g in range(ngroups):
        colbase.append(colbase[-1] + gchunks[g])
    runs_by_g = {g: [] for g in range(ngroups)}
    for (g, p, base, n_real) in rt["runs"]:
        runs_by_g[g].append((p, base, n_real))
    wchunks = rt["wchunks"]                # wi -> [(dcol, gcol, half)]

    ESIZE = NPHASE * D if elem512 else 2 * D   # gather element (bf16 elems)

    Relu = mybir.ActivationFunctionType.Relu
    Exp = mybir.ActivationFunctionType.Exp
    add_op = mybir.AluOpType.add
    eq_op = mybir.AluOpType.is_equal

    with tile.TileContext(nc) as tc:
        import contextlib
        with contextlib.ExitStack() as ctx:
            cpool = ctx.enter_context(tc.tile_pool(name="const", bufs=1))
            ypool = ctx.enter_context(tc.tile_pool(name="yown", bufs=1))

            fake_gt = None
            if skip_gather:
                fake_gt = cpool.tile([WIN, 4096], bf16)
                nc.scalar.memzero(fake_gt[:])

            idx_sb = (cpool.tile([128, tot // 16], i16)
                      if need_idx1 else None)
            if cc_chunks > 1:
                idx2_sb = cpool.tile([128, tot // 16], i16)
            else:
                idx2_sb = idx_sb
            dst_sb = cpool.tile([WIN, rt["ndcol"]], f32)
            iota_sb = cpool.tile([WIN, WIN], bf16)
            if not l1red:
                hTo_sb = cpool.tile([D, nlocp], f32)
            W1_sb = cpool.tile([2 * D, HID], f32)
            b1_sb = cpool.tile([HID, 1], f32)
            W2_sb = cpool.tile([HID, D], f32)
            b2_sb = cpool.tile([WIN, D], f32)
            yown = ypool.tile([WIN, nw * D], f32)

            if need_idx1:
                nc.sync.dma_start(idx_sb[:], idxd[:])
            if cc_chunks > 1:
                nc.sync.dma_start(idx2_sb[:], idx2d[:])
            nc.sync.dma_start(dst_sb[:], dstd[:])
            nc.sync.dma_start(iota_sb[:], iotad[:])
            if not l1red:
                nc.sync.dma_start(hTo_sb[:], hTo[:])
            nc.sync.dma_start(W1_sb[:], W1d[:])
            nc.sync.dma_start(b1_sb[:], b1d[:])
            nc.sync.dma_start(W2_sb[:], W2d[:])
            nc.sync.dma_start(b2_sb[:], b2d[:])

            MAXIDX = maxidx  # default 1024: 64 desc/engine x 16 engines/packet
            qctr = [0]      # round-robin SWDGE queue (4 Q7 core pairs)

            def issue_gathers(g, gt, table, isb=None):
                if skip_gather:
                    return
                isb = idx_sb if isb is None else isb
                # fetch whole padded chunk grids: skipped (negative) indices
                # would leave stale SBUF rows, and 0 * NaN = NaN defeats the
                # one-hot masking. Adjacent phase pairs share the table half
                # (j = p>>1) and sit at consecutive positions -> one gather;
                # with elem512 (whole super-row) all phases merge.
                merged = []                     # [j, base, n_pad]
                for (p, base, n_real) in runs_by_g[g]:
                    n_pad = (n_real + WIN - 1) // WIN * WIN
                    jkey = 0 if elem512 else p >> 1
                    if merged and merged[-1][0] == jkey:
                        merged[-1][2] += n_pad
                    else:
                        merged.append([jkey, base, n_pad])
                for (j, base, n_pad) in merged:
                    if n_pad == 0:
                        continue
                    in_ap = (table[:] if elem512
                             else table[:, j * 2 * D:(j + 1) * 2 * D])
                    for s0 in range(0, n_pad, MAXIDX):
                        nr = min(MAXIDX, n_pad - s0)
                        ni = nr
                        nco = (ni + WIN - 1) // WIN     # out chunk cols
                        c0 = base // WIN - colbase[g] + s0 // WIN
                        oo = base + s0
                        nc.gpsimd.dma_gather(
                            out_ap=gt[:, c0 * ESIZE:(c0 + nco) * ESIZE]
                            .rearrange("p (c f) -> p c f", f=ESIZE),
                            in_ap=in_ap,
                            idxs_ap=isb[:, oo // 16: (oo + ni) // 16],
                            num_idxs=ni,
                            num_idxs_reg=nr,
                            elem_size=ESIZE,
                            elem_step=NPHASE * D,
                            single_packet=single_packet,
                            queue_num=qctr[0] % 4,
                        )
                        qctr[0] += 1

            def chunk_src(gt, i, lcol, ph):
                if skip_gather:
                    return fake_gt[:, (i % 63) * 64:(i % 63) * 64 + D]
                c0 = lcol * ESIZE + (ph if elem512 else ph & 1) * D
                return gt[:, c0:c0 + D]

            # ---------------- stage A: layer 1 ----------------
            # l1red streams have no chunk-map ties, so stage A may batch
            # more windows per DMA than the gather GROUP; the cc-chunk
            # trigger below still fires because chunk boundaries are
            # multiples of both group sizes.
            l1g = (l1group or group) if l1red else group
            ngl1 = math.ceil(nw / l1g)
            with contextlib.ExitStack() as sa:
                gpool = sa.enter_context(tc.tile_pool(name="gatherA", bufs=gbufs))
                ohpool = sa.enter_context(tc.tile_pool(name="ohA", bufs=16))
                aggpool = sa.enter_context(tc.tile_pool(name="aggT", bufs=4))
                xpool = sa.enter_context(tc.tile_pool(name="x1", bufs=4))
                if not l1red:
                    psA = sa.enter_context(
                        tc.tile_pool(name="psA", bufs=3, space="PSUM"))
                psB = sa.enter_context(
                    tc.tile_pool(name="psB", bufs=4 if l1red else 2,
                                 space="PSUM"))
                psC = sa.enter_context(
                    tc.tile_pool(name="psC", bufs=2, space="PSUM"))

                for g in range(ngl1):
                    ws = list(range(g * l1g, min((g + 1) * l1g, nw)))
                    if l1red:
                        c0w, c1w = int(l1off[ws[0]]), int(l1off[ws[-1] + 1])
                        gt = gpool.tile([128, c1w - c0w], bf16, tag="gbuf")
                        nc.sync.dma_start(gt[:], l1rd[:, c0w:c1w])
                    else:
                        gt = gpool.tile([WIN, gchunks[g] * D], bf16,
                                        tag="gbuf")
                        ca = colbase[g]
                        nc.sync.dma_start(
                            gt[:], l1sd[:, ca * D:(ca + gchunks[g]) * D])
                    for wi in ws:
                        if skip_compute:
                            ybf = xpool.tile([WIN, D], bf16, tag="ybf")
                            nc.scalar.copy(ybf[:], gt[:, 0:D])
                            nc.scalar.copy(yown[:, wi * D:(wi + 1) * D],
                                           b2_sb[:])
                            rows = last_rows if wi == nw - 1 else WIN
                            nc.sync.dma_start(
                                cc_in[wi * WIN: wi * WIN + rows, :],
                                ybf[:rows, :ccd])
                            continue
                        x1 = xpool.tile([HID, WIN], f32)
                        if l1red:
                            ns = int(nslot[wi])
                            off = int(l1off[wi]) - c0w
                            red = aggpool.tile([128, D], f32)
                            nc.vector.tensor_reduce(
                                red[:],
                                gt[:, off:off + D * ns]
                                .rearrange("p (d j) -> p d j", j=ns),
                                mybir.AxisListType.X, add_op)
                            ps2a = psB.tile([HID, D], f32, tag="ps2")
                            ps2b = psB.tile([HID, D], f32, tag="ps2")
                            nc.tensor.matmul(ps2a[:], W1_sb[0:D, :],
                                             red[0:D, :])
                            nc.tensor.matmul(ps2b[:], W1_sb[D:2 * D, :],
                                             red[D:2 * D, :])
                            nc.scalar.activation(x1[:, 0:D], ps2a[:], Relu,
                                                 bias=b1_sb[:, 0:1])
                            nc.scalar.activation(x1[:, D:2 * D], ps2b[:],
                                                 Relu, bias=b1_sb[:, 0:1])
                        else:
                            chunks = wchunks[wi]
                            ps = psA.tile([D, WIN], f32)
                            for i, (dcol, gcol, half) in enumerate(chunks):
                                lcol = gcol - colbase[g]
                                oh = ohpool.tile([WIN, WIN], bf16)
                                nc.vector.tensor_scalar(
                                    oh[:], iota_sb[:],
                                    dst_sb[:, dcol:dcol + 1],
                                    None, eq_op)
                                lhsT = gt[:, lcol * D:lcol * D + D]
                                nc.tensor.matmul(
                                    ps[:], lhsT, oh[:],
                                    start=(i == 0),
                                    stop=(i == len(chunks) - 1))
                            aggT = aggpool.tile([D, WIN], f32)
                            nc.vector.tensor_tensor(
                                aggT[:], ps[:],
                                hTo_sb[:, wi * WIN:(wi + 1) * WIN],
                                add_op)
                            ps2 = psB.tile([HID, WIN], f32)
                            nc.tensor.matmul(ps2[:], W1_sb[0:D, :], aggT[:])
                            nc.scalar.activation(x1[:], ps2[:], Relu,
                                                 bias=b1_sb[:, 0:1])
                        ps3 = psC.tile([WIN, D], f32)
                        nc.tensor.matmul(ps3[:], x1[:], W2_sb[:])
                        nc.scalar.copy(yown[:, wi * D:(wi + 1) * D], ps3[:])
                        ybf = xpool.tile([WIN, D], bf16, tag="ybf")
                        nc.scalar.copy(ybf[:], ps3[:])
                        rows = last_rows if wi == nw - 1 else WIN
                        nc.sync.dma_start(
                            cc_in[wi * WIN: wi * WIN + rows, :],
                            ybf[:rows, :ccd])
                    if do_cc and cc_chunks > 1:
                        last_w = ws[-1]
                        for q, (a, b) in enumerate(rt["cc_bounds"]):
                            bw = (b + WIN - 1) // WIN - 1   # last window of q
                            if bw == last_w:
                                nc.gpsimd.collective_compute(
                                    "AllGather", mybir.AluOpType.bypass,
                                    replica_groups=[list(range(CORES))],
                                    ins=[cc_in[a:b, :]],
                                    outs=[y40[CORES * a:CORES * b, :]
                                          if cc40 else y4[2 * a:2 * b, :]])
                                if cc40:
                                    expand_y40(a, b)

            # ---------------- all-gather of y ----------------
            if do_cc and cc_chunks == 1:
                nc.gpsimd.collective_compute(
                    "AllGather", mybir.AluOpType.bypass,
                    replica_groups=[list(range(CORES))],
                    ins=[cc_in.ap().opt()],
                    outs=[y40.ap().opt() if cc40 else y4.ap().opt()])
                if cc40:
                    expand_y40(0, nloc)

            # ---------------- stage C: layer 2 ----------------
            with contextlib.ExitStack() as sc:
                gpool = sc.enter_context(tc.tile_pool(name="gatherC", bufs=gbufs))
                ohpool = sc.enter_context(tc.tile_pool(name="ohC", bufs=16))
                spool = sc.enter_context(tc.tile_pool(name="smax", bufs=4))
                opool = sc.enter_context(tc.tile_pool(name="outp", bufs=3))
                psD = sc.enter_context(
                    tc.tile_pool(name="psD", bufs=4, space="PSUM"))

                for g in range(ngroups):
                    gt = gpool.tile([WIN, gchunks[g] * ESIZE], bf16, tag="gbufC")
                    issue_gathers(g, gt, y4, isb=idx2_sb)
                    for wi in group_windows(g):
                        if skip_compute:
                            lcol0 = wchunks[wi][0][1] - colbase[g]
                            o = opool.tile([WIN, C], f32)
                            nc.scalar.copy(
                                o[:], gt[:, lcol0 * ESIZE:lcol0 * ESIZE + C])
                            rows = last_rows if wi == nw - 1 else WIN
                            nc.sync.dma_start(
                                outd[wi * WIN: wi * WIN + rows, :],
                                o[:rows, :])
                            continue
                        chunks = wchunks[wi]
                        ps = psD.tile([WIN, D], f32)
                        for i, (dcol, gcol, half) in enumerate(chunks):
                            lcol = gcol - colbase[g]
                            oh = ohpool.tile([WIN, WIN], bf16)
                            nc.vector.tensor_scalar(
                                oh[:], iota_sb[:], dst_sb[:, dcol:dcol + 1],
                                None, eq_op)
                            nc.tensor.matmul(
                                ps[:], oh[:], chunk_src(gt, i, lcol, half),
                                start=(i == 0), stop=(i == len(chunks) - 1))
                        t1 = spool.tile([WIN, D], f32, tag="t1")
                        nc.vector.tensor_tensor(
                            t1[:], ps[:], yown[:, wi * D:(wi + 1) * D], add_op)
                        t2 = spool.tile([WIN, D], f32, tag="t2")
                        nc.vector.tensor_tensor(t2[:], t1[:], b2_sb[:], add_op)
                        mx = spool.tile([WIN, 1], f32, tag="mx")
                        nc.vector.tensor_reduce(
                            mx[:], t2[:, :C], mybir.AxisListType.X,
                            mybir.AluOpType.max, negate=True)
                        e = spool.tile([WIN, C], f32, tag="e")
                        nc.scalar.activation(e[:], t2[:, :C], Exp,
                                             bias=mx[:, 0:1])
                        sm = spool.tile([WIN, 1], f32, tag="sm")
                        nc.vector.tensor_reduce(
                            sm[:], e[:], mybir.AxisListType.X, add_op)
                        ri = spool.tile([WIN, 1], f32, tag="ri")
                        nc.vector.reciprocal(ri[:], sm[:])
                        o = opool.tile([WIN, C], f32)
                        nc.vector.tensor_scalar_mul(o[:], e[:], ri[:, 0:1])
                        rows = last_rows if wi == nw - 1 else WIN
                        nc.sync.dma_start(
                            outd[wi * WIN: wi * WIN + rows, :], o[:rows, :])

    nc.finalize()
    return nc


# ----------------------------------------------------------------------------
# Entry point
# ----------------------------------------------------------------------------

def _prepare_inputs(node_embeddings, adjacency_lists, W1, b1, W2, b2, rt,
                    rt1=None, cc40=False):
    n, d = node_embeddings.shape
    nloc, nw = rt["nloc"], rt["nw"]
    nlocp = nw * WIN
    bf = np.float16
    h = np.ascontiguousarray(node_embeddings, np.float32)
    h_bf = h.astype(bf)
    W2p = np.zeros((HID, D), np.float32)
    W2p[:, :C] = W2
    b2b = np.tile(np.pad(b2.astype(np.float32), (0, D - C)), (WIN, 1))
    iota = np.tile(np.arange(WIN, dtype=np.float32), (WIN, 1))
    in_maps = []
    for k in range(CORES):
        if rt1 is not None:
            l1_entries = {"l1r": rt1["streams"][k]}
        else:
            hTo = np.zeros((d, nlocp), np.float32)
            hTo[:, :nloc] = h[k * nloc:(k + 1) * nloc].T
            # pre-gathered layer-1 stream: [tot,64] -> [WIN, nch*D] chunkwise
            l1s = (h_bf[rt["srcs"][k]].reshape(-1, WIN, D)
                   .transpose(1, 0, 2).reshape(WIN, -1))
            l1_entries = {"l1s": np.ascontiguousarray(l1s), "hTo": hTo}
        in_maps.append({
            **l1_entries,
            "W1": np.ascontiguousarray(np.vstack([W1, W1]), np.float32),
            "b1": np.ascontiguousarray(b1, np.float32).reshape(HID, 1),
            "W2p": W2p,
            "b2b": b2b,
            **({"idx": np.tile(rt["idx"][k].reshape(-1, 16).T,
                               (8, 1)).copy()}
               if (rt["cc_chunks"] == 1 or rt1 is None) else {}),
            **({"idx2": np.tile(rt["idx2"][k].reshape(-1, 16).T,
                                (8, 1)).copy()}
               if rt["cc_chunks"] > 1 else {}),
            "dstf": np.ascontiguousarray(rt["dst"][k].T),
            "iota": iota.astype(bf),
            **({"y4z": np.zeros((CORES * nloc // NPHASE, NPHASE * D), bf)}
               if cc40 else {}),
            "out": np.zeros((nloc, C), np.float32),
        })
    return in_maps


_CACHE = {}


def _get_program(n_nodes, rt_sig, rt):
    key = (n_nodes, rt_sig)
    if key not in _CACHE:
        _CACHE[key] = build_program(n_nodes, rt)
    return _CACHE[key]


def build_all(node_embeddings, adjacency_lists, W1, b1, W2, b2,
              cache=True, **build_flags):
    """Route edges, build (cached) program, prepare per-core inputs."""
    n = node_embeddings.shape[0]
    src = np.asarray(adjacency_lists)[:, 0]
    dst = np.asarray(adjacency_lists)[:, 1]
    rt = route_edges(src, dst, n,
                     cc_chunks=build_flags.get("cc_chunks", 1),
                     group=build_flags.get("group", GROUP),
                     cc_geo=build_flags.get("cc_geo", False))
    rt1 = None
    if build_flags.get("l1red", False):
        h_bf = (np.ascontiguousarray(node_embeddings, np.float32)
                .astype(np.float16))
        rt1 = route_l1(src, dst, h_bf, n)
    if cache:
        rt_sig = (rt["tot"], tuple(rt["S"].tolist()),
                  tuple(rt1["nslot"].tolist()) if rt1 else None,
                  tuple(sorted(build_flags.items())))
        key = (n, rt_sig)
        if key not in _CACHE:
            _CACHE[key] = build_program(n, rt, rt1, **build_flags)
        nc = _CACHE[key]
    else:
        nc = build_program(n, rt, rt1, **build_flags)
    in_maps = _prepare_inputs(node_embeddings, adjacency_lists,
                              W1, b1, W2, b2, rt, rt1=rt1,
                              cc40=build_flags.get("cc40", False))
    return nc, in_maps, rt


# tuned build configuration used by kernel()
KERNEL_FLAGS = {"cc40": False, "cc_chunks": 4, "single_packet": False,
                "maxidx": 4096, "scratch": 32768, "l1red": True,
                "gbufs": 4}


def kernel(node_embeddings, adjacency_lists, W1, b1, W2, b2, trace=False):
    import sys
    if "/opt/trn_rl_repo" not in sys.path:
        sys.path.insert(0, "/opt/trn_rl_repo")
    from concourse import bass_utils

    nc, in_maps, rt = build_all(node_embeddings, adjacency_lists,
                                W1, b1, W2, b2, **KERNEL_FLAGS)
    res = bass_utils.run_bass_kernel_spmd(
        nc, in_maps, core_ids=list(range(CORES)), trace=trace)
    out = np.concatenate([res.results[k]["out"] for k in range(CORES)], axis=0)
    kernel.last_result = res
    kernel.last_nc = nc
    kernel.last_in_maps = in_maps
    return out

